# revision 17
# baseline (speedup 1.0000x reference)
"""BitLinear forward (RMSNorm + absmean ternary weight quant + absmax int8
activation quant + scaled matmul), tensor-parallel over 8 NeuronCores.

Sharding: column-parallel linear — weight rows (out_features) split 8 ways;
x is replicated; alpha (global mean |w|) via a tiny AllReduce; each core
computes y[:, shard] and the host concatenates.

Exactness: quantized activations are integers in [-127, 127] and quantized
weights are in {-1, 0, 1}, so the matmul runs in bf16 (lhsT) x fp8e4 (rhs)
with fp32 PSUM accumulation and is bit-exact (all partial sums < 2^24).
"""

import numpy as np

import concourse.bass as bass
import concourse.mybir as mybir
import concourse.tile as tile
from concourse.bass_utils import run_bass_kernel_spmd

# The walrus build available here rejects instructions carrying more than one
# attached sync-wait ("Too many sync wait commands"), which Tile emits
# routinely.  Hoist extras onto single-wait NoOps on the same engine —
# engine streams are in-order so wait-then-issue is equivalent.
MAX_ATTACHED_WAITS = 1


def _split_sync_waits(nc, max_waits=MAX_ATTACHED_WAITS):
    nhoisted = 0
    for f in nc.m.functions:
        for blk in f.blocks:
            out = []
            changed = False
            for inst in blk.instructions:
                si = inst.sync_info
                if si is not None and len(si.on_wait) > max_waits:
                    waits = list(si.on_wait)
                    for wt in waits[max_waits:]:
                        out.append(
                            mybir.InstNoOp(
                                name=f"syncsplit-{nc.next_id()}",
                                ins=[],
                                outs=[],
                                engine=inst.engine,
                                sync_info=mybir.SyncInfo(
                                    on_wait=[wt], on_update=[]
                                ),
                                bass_nofuse=True,
                            )
                        )
                        nhoisted += 1
                    inst.sync_info = mybir.SyncInfo(
                        on_wait=waits[:max_waits], on_update=list(si.on_update)
                    )
                    changed = True
                out.append(inst)
            if changed:
                blk.instructions = out
    return nhoisted

F32 = mybir.dt.float32
BF16 = mybir.dt.bfloat16
FP8 = mybir.dt.float8e4

MAGIC = 1.5 * 2.0**23  # add/sub rounds f32 to nearest int (ties to even)
EPS = 1e-6

N_CORES = 8
IN_FEATURES = 4096
OUT_FEATURES = 16384
BATCH_SHAPE = (2, 2048)


def build(T, K, O, n_cores):
    """One-core SPMD program: x[T,K] f32, w[O,K] f32 shard, nw[1,K] -> y[T,O]."""
    TT, KT, OT = T // 128, K // 128, O // 128
    OBN = max(1, O // 512)  # number of 512-wide output column blocks
    OBW = O // OBN
    assert OBW <= 512

    nc = bass.Bass(
        "TRN2", target_bir_lowering=False, debug=False, num_devices=n_cores
    )
    x = nc.dram_tensor("x", [T, K], F32, kind="ExternalInput")
    w = nc.dram_tensor("w", [O, K], F32, kind="ExternalInput")
    nw = nc.dram_tensor("nw", [1, K], F32, kind="ExternalInput")
    y = nc.dram_tensor("y", [T, O], F32, kind="ExternalOutput")

    inv_count = 1.0 / (O * n_cores * K)  # power of two for real sizes

    with tile.TileContext(nc) as tc:
        with (
            tc.tile_pool(name="const", bufs=1) as cpool,
            tc.tile_pool(name="wres", bufs=1) as wres,
            tc.tile_pool(name="stat", bufs=4) as spool,
            tc.tile_pool(name="psum", bufs=8, space="PSUM") as ps,
            tc.tile_pool(name="dram", bufs=1, space="DRAM") as dram,
        ):
            # ---- constants ----
            negmagic = cpool.tile([128, 1], F32, tag="negmagic")
            nc.vector.memset(negmagic[:], -MAGIC)
            epsb = cpool.tile([128, 1], F32, tag="epsb")
            nc.vector.memset(epsb[:], EPS)
            ones_row = cpool.tile([1, 128], F32, tag="ones_row")
            nc.vector.memset(ones_row[:], 1.0)
            ones_col = cpool.tile([128, 1], F32, tag="ones_col")
            nc.vector.memset(ones_col[:], 1.0)
            alpha_bc = cpool.tile([128, 1], F32, tag="alpha_bc")
            inv_alpha_bc = cpool.tile([128, 1], F32, tag="inv_alpha_bc")

            nw_rep = cpool.tile([128, K], F32, tag="nw_rep")

            # resident transposed ternary weights, fp8 (exact for -1/0/1)
            # layout: [128, KT*O]; block kt spans cols [kt*O, (kt+1)*O)
            wqT = wres.tile([128, KT * O], FP8, tag="wqT")

            # ---- phase W1: global alpha = max(mean |w|, 1e-10) ----
            with tc.tile_pool(name="wph", bufs=1) as wph:
                # replicate norm_weight to all 128 partitions (log-doubling)
                nc.gpsimd.dma_start(nw_rep[0:1, :], nw.ap())
                p = 1
                while p < 128:
                    nc.gpsimd.dma_start(nw_rep[p : 2 * p, :], nw_rep[0:p, :])
                    p *= 2

                wsum = wph.tile([128, OT], F32, tag="wsum")
                with tc.tile_pool(name="wp", bufs=3) as wp, tc.tile_pool(
                    name="wp2", bufs=2
                ) as wp2:
                    for ot in range(OT):
                        wt = wp.tile([128, K], F32, tag="wt")
                        nc.gpsimd.dma_start(wt[:], w[ot * 128 : (ot + 1) * 128, :])
                        absw = wp2.tile([128, K], BF16, tag="wscr")
                        nc.scalar.activation(
                            absw[:],
                            wt[:],
                            mybir.ActivationFunctionType.Abs,
                            accum_out=wsum[:, ot : ot + 1],
                        )

                    # reduce to scalar: free-dim sum then partition sum via matmul
                    wred = spool.tile([128, 1], F32, tag="wred")
                    nc.vector.reduce_sum(wred[:], wsum[:], axis=mybir.AxisListType.X)
                    pss = ps.tile([1, 1], F32, tag="ps")
                    nc.tensor.matmul(
                        pss[:], wred[:], ones_col[:], start=True, stop=True
                    )
                    total_sb = spool.tile([1, 8], F32, tag="total_sb")
                    nc.vector.memset(total_sb[:], 0.0)
                    nc.vector.tensor_copy(total_sb[:, 0:1], pss[:])

                    # AllReduce the partial |w| sums
                    cc_in = dram.tile([1, 8], F32, tag="cc_in")
                    cc_out = dram.tile([1, 8], F32, tag="cc_out")
                    nc.gpsimd.dma_start(cc_in[:], total_sb[:])
                    nc.gpsimd.collective_compute(
                        "AllReduce",
                        mybir.AluOpType.add,
                        replica_groups=[list(range(n_cores))],
                        ins=[cc_in.opt()],
                        outs=[cc_out.opt()],
                    )
                    gtot = spool.tile([1, 1], F32, tag="gtot")
                    nc.gpsimd.dma_start(gtot[:], cc_out[:, 0:1])
                    alpha_s = spool.tile([1, 1], F32, tag="alpha_s")
                    nc.vector.tensor_scalar(
                        out=alpha_s[:],
                        in0=gtot[:],
                        scalar1=inv_count,
                        scalar2=1e-10,
                        op0=mybir.AluOpType.mult,
                        op1=mybir.AluOpType.max,
                    )
                    # broadcast alpha and 1/alpha to all 128 partitions
                    nc.vector.tensor_copy(alpha_bc[0:1, :], alpha_s[:])
                    inv_alpha_s = spool.tile([1, 1], F32, tag="inv_alpha_s")
                    nc.vector.reciprocal(inv_alpha_s[:], alpha_s[:])
                    nc.vector.tensor_copy(inv_alpha_bc[0:1, :], inv_alpha_s[:])
                    p = 1
                    while p < 128:
                        nc.gpsimd.dma_start(
                            alpha_bc[p : 2 * p, :], alpha_bc[0:p, :]
                        )
                        nc.gpsimd.dma_start(
                            inv_alpha_bc[p : 2 * p, :], inv_alpha_bc[0:p, :]
                        )
                        p *= 2

                    # ---- phase W2: quantize + transpose weights ----
                    for ot in range(OT):
                        wt = wp.tile([128, K], F32, tag="wt")
                        nc.gpsimd.dma_start(wt[:], w[ot * 128 : (ot + 1) * 128, :])
                        wdiv = wp.tile([128, K], F32, tag="wt")
                        # (w / alpha) + MAGIC : rounds to nearest int
                        nc.vector.tensor_scalar(
                            out=wdiv[:],
                            in0=wt[:],
                            scalar1=inv_alpha_bc[:],
                            scalar2=MAGIC,
                            op0=mybir.AluOpType.mult,
                            op1=mybir.AluOpType.add,
                        )
                        w2 = wp.tile([128, K], F32, tag="wt")
                        nc.vector.tensor_scalar(
                            out=w2[:],
                            in0=wdiv[:],
                            scalar1=MAGIC,
                            scalar2=-1.0,
                            op0=mybir.AluOpType.subtract,
                            op1=mybir.AluOpType.max,
                        )
                        wqb = wp2.tile([128, K], BF16, tag="wscr")
                        nc.vector.tensor_scalar_min(wqb[:], w2[:], 1.0)
                        # transpose all KT 128x128 blocks in one DMA-transpose
                        wqTs = wp2.tile([128, KT * 128], BF16, tag="wqTs")
                        nc.sync.dma_start(
                            wqTs[:].rearrange("p (j f) -> p j f", f=128),
                            wqb[:].rearrange("p (j f) -> p j f", f=128),
                            transpose=True,
                        )
                        # scatter-convert bf16 -> fp8 resident wqT
                        nc.vector.tensor_copy(
                            wqT[:].rearrange("p (j o) -> p j o", o=O)[
                                :, :, ot * 128 : (ot + 1) * 128
                            ],
                            wqTs[:].rearrange("p (j f) -> p j f", f=128),
                        )

            # ---- main loop over token tiles ----
            with tc.tile_pool(name="xa", bufs=3) as xa, tc.tile_pool(
                name="xb", bufs=2
            ) as xb, tc.tile_pool(name="xc", bufs=1) as xc:
                for tt in range(TT):
                    xin = xa.tile([128, K], F32, tag="xin")
                    nc.gpsimd.dma_start(xin[:], x[tt * 128 : (tt + 1) * 128, :])

                    # sum of squares (for rms) on ACT; u = x*nw and absmax on DVE
                    x2 = xc.tile([128, K], BF16, tag="x2")
                    ss = spool.tile([128, 1], F32, tag="ss")
                    nc.scalar.activation(
                        x2[:],
                        xin[:],
                        mybir.ActivationFunctionType.Square,
                        accum_out=ss[:],
                    )
                    u = xb.tile([128, K], F32, tag="u", bufs=1)
                    nc.vector.tensor_mul(u[:], xin[:], nw_rep[:])
                    graw = spool.tile([128, 1], F32, tag="graw")
                    nc.vector.tensor_reduce(
                        graw[:],
                        u[:],
                        axis=mybir.AxisListType.X,
                        op=mybir.AluOpType.max,
                        apply_absolute_value=True,
                    )
                    g = spool.tile([128, 1], F32, tag="g")
                    nc.vector.tensor_scalar_max(g[:], graw[:], 1e-10)

                    # per-token scales
                    invg = spool.tile([128, 1], F32, tag="invg")
                    nc.vector.reciprocal(invg[:], g[:])
                    s127 = spool.tile([128, 1], F32, tag="s127")
                    nc.vector.tensor_scalar_mul(s127[:], invg[:], 127.0)
                    rms = spool.tile([128, 1], F32, tag="rms")
                    nc.scalar.activation(
                        rms[:],
                        ss[:],
                        mybir.ActivationFunctionType.Sqrt,
                        bias=epsb[:],
                        scale=1.0 / K,
                    )
                    invrms = spool.tile([128, 1], F32, tag="invrms")
                    nc.vector.reciprocal(invrms[:], rms[:])
                    gor = spool.tile([128, 1], F32, tag="gor")
                    nc.vector.tensor_mul(gor[:], g[:], invrms[:])
                    sy = spool.tile([128, 1], F32, tag="sy")
                    nc.vector.tensor_scalar(
                        out=sy[:],
                        in0=gor[:],
                        scalar1=alpha_bc[:],
                        scalar2=1.0 / 127.0,
                        op0=mybir.AluOpType.mult,
                        op1=mybir.AluOpType.mult,
                    )

                    # quantize: round(u * 127/g) via magic add/sub
                    q1 = xa.tile([128, K], F32, tag="xin")
                    nc.vector.tensor_scalar(
                        out=q1[:],
                        in0=u[:],
                        scalar1=s127[:],
                        scalar2=MAGIC,
                        op0=mybir.AluOpType.mult,
                        op1=mybir.AluOpType.add,
                    )
                    xq = xb.tile([128, K], BF16, tag="xq")
                    nc.scalar.activation(
                        xq[:],
                        q1[:],
                        mybir.ActivationFunctionType.Identity,
                        bias=negmagic[:],
                    )

                    # transpose all KT blocks in one DMA-transpose call
                    xqT = xb.tile([128, K], BF16, tag="xqT")
                    nc.sync.dma_start(
                        xqT[:].rearrange("p (j f) -> p j f", f=128),
                        xq[:].rearrange("p (j f) -> p j f", f=128),
                        transpose=True,
                    )

                    # matmul: psum[t, ob] += xq_block @ wqT_block
                    psums = [
                        ps.tile([128, OBW], F32, tag="ps", name=f"psum_{tt}_{ob}")
                        for ob in range(OBN)
                    ]
                    for kt in range(KT):
                        lhsT = xqT[:, kt * 128 : (kt + 1) * 128]
                        for ob in range(OBN):
                            nc.tensor.matmul(
                                psums[ob][:],
                                lhsT,
                                wqT[:, kt * O + ob * OBW : kt * O + (ob + 1) * OBW],
                                start=(kt == 0),
                                stop=(kt == KT - 1),
                            )

                    # epilogue: scale by alpha*gamma/127 and store
                    osb = xb.tile([128, O], F32, tag="osb")
                    for ob in range(OBN):
                        nc.vector.tensor_scalar_mul(
                            osb[:, ob * OBW : (ob + 1) * OBW], psums[ob][:], sy[:]
                        )
                    nc.gpsimd.dma_start(y[tt * 128 : (tt + 1) * 128, :], osb[:])

    return nc


_nc_cache = {}


def _get_nc(T, K, O, n_cores):
    key = (T, K, O, n_cores)
    if key not in _nc_cache:
        nc = build(T, K, O, n_cores)
        _split_sync_waits(nc)  # HW-only fixup; CoreSim rejects bare NoOps
        _nc_cache[key] = nc
    return _nc_cache[key]


def kernel(x: np.ndarray, weight: np.ndarray, norm_weight: np.ndarray) -> np.ndarray:
    B, S, K = x.shape
    T = B * S
    Ofull, _ = weight.shape
    O = Ofull // N_CORES

    nc = _get_nc(T, K, O, N_CORES)

    xf = np.ascontiguousarray(x.reshape(T, K).astype(np.float32, copy=False))
    nwf = np.ascontiguousarray(norm_weight.reshape(1, K).astype(np.float32, copy=False))
    in_maps = [
        {
            "x": xf,
            "w": np.ascontiguousarray(weight[i * O : (i + 1) * O]),
            "nw": nwf,
        }
        for i in range(N_CORES)
    ]
    res = run_bass_kernel_spmd(nc, in_maps, list(range(N_CORES))).results
    y = np.concatenate([res[i]["y"] for i in range(N_CORES)], axis=1)
    return y.reshape(B, S, Ofull)
